# revision 1
# baseline (speedup 1.0000x reference)
# Trainium2 Bass kernel for nn_MultiHeadAttention_71674414235938
#
# MHA with a cross-modal additive bias gathered from a 3x3 table and a causal
# mask, B=1, S=2048, HID=1024, H=16 heads of D=64.
#
# Sharding: tensor-parallel over heads. 2 heads per core (dq slice of 128).
# Each core computes q/k/v projections for its heads, head-local attention,
# and a partial output ctx_c @ Wo[:, c*128:(c+1)*128].T which the host sums.
#
# Device-side layout choices:
#   * scores are computed TRANSPOSED: sT[j, i] = k[j]·q[i] (j on partitions),
#     so softmax-denominators and the attn@V contraction both run without any
#     on-chip transposes:  ctxT[d, i] = sum_j v'[j, d] * attnT[j, i]  with
#     lhsT = v' (natural layout) and rhs = attnT (as produced).
#   * the 3x3 cross-modal bias is rank-3:  bias = (onehot(m) @ cmw) @ onehot(m).T
#     so it is folded into the scores matmul by appending 3 rows (U.T to the
#     q side, R.T to the k side), K = 64+3 = 67.
#   * softmax runs without max-subtraction: scores are O(+-6) here, exp is
#     safely in fp32 range.
#   * a ones-column appended to v makes the PE accumulate the softmax
#     denominator into ctxT row 64; normalization = reciprocal of that row +
#     a partition-broadcast (gpsimd mid-kernel, PE ones-matmul at the tail)
#     + multiply on the way out of PSUM.
#   * causal structure: score blocks entirely above the diagonal are skipped;
#     diagonal staircase blocks are masked multiplicatively after exp.
#
# Schedule (the big wins over the first working version):
#   * weights are host-permuted to a [128, kc*m] layout so every input DMA is
#     contiguous at line rate; sync-queue order wq, xk0, wk, xk1.. lets the
#     first matmul start ~3us in instead of ~15us.
#   * all 8 q/k chains accumulate together, kc-ordered, across all 8 PSUM
#     banks, so the PE consumes each x chunk the moment it lands.
#   * ctx accumulation steps trail the score/exp stream inside each phase
#     (open PSUM accumulation groups interleave fine), so no bulk ctx drain
#     is left for the end.
#   * per-head score PSUM tiles + exp; q-side PSUM->SBUF copies go on the
#     scalar engine, k-side on vector, halving the copy burst after the
#     projection phase.

import math

import numpy as np
import ml_dtypes

B, S, HID, H, D = 1, 2048, 1024, 16, 64
NCORES = 8
HPC = H // NCORES          # heads per core = 2
DPC = HPC * D              # head-dim columns per core = 128
KC = HID // 128            # contraction chunks = 8
NIC = S // 512             # 512-wide i-chunks = 4
NJB = S // 128             # 128-tall j-blocks = 16

BF16 = ml_dtypes.bfloat16

_CACHE = {}


def _build(causal: bool, has_bq: bool, has_bk: bool, has_bv: bool):
    from contextlib import ExitStack

    import concourse.bass as bass
    import concourse.bacc as bacc
    import concourse.mybir as mybir
    import concourse.tile as tile

    fp32 = mybir.dt.float32
    f32r = mybir.dt.float32r
    bf16 = mybir.dt.bfloat16
    Exp = mybir.ActivationFunctionType.Exp
    Copy = mybir.ActivationFunctionType.Copy

    nc = bacc.Bacc()

    xT = nc.declare_dram_parameter("xT", [HID, S], bf16, isOutput=False)
    # host-permuted: row p holds Wq.T[kc*128+p, :] for all kc, contiguous
    wqL = nc.declare_dram_parameter("wqL", [128, KC * DPC], bf16, isOutput=False)
    wkL = nc.declare_dram_parameter("wkL", [128, KC * DPC], bf16, isOutput=False)
    wvL = nc.declare_dram_parameter("wvL", [128, KC * DPC], bf16, isOutput=False)
    woT = nc.declare_dram_parameter("woT", [DPC, HID], bf16, isOutput=False)
    uT = nc.declare_dram_parameter("uT", [4, S], bf16, isOutput=False)
    rT = nc.declare_dram_parameter("rT", [4, S], bf16, isOutput=False)
    if has_bq:
        bq = nc.declare_dram_parameter("bq", [DPC, 1], fp32, isOutput=False)
    if has_bk:
        bk = nc.declare_dram_parameter("bk", [DPC, 1], fp32, isOutput=False)
    if has_bv:
        bv = nc.declare_dram_parameter("bv", [1, DPC], fp32, isOutput=False)
    if not causal:
        maskT = nc.declare_dram_parameter("maskT", [S, S], bf16, isOutput=False)
    out = nc.declare_dram_parameter("out", [S, HID], bf16, isOutput=True)

    with tile.TileContext(nc) as tc, ExitStack() as ctx:
        pp = ctx.enter_context(tc.tile_pool(name="persist", bufs=1))

        # -- input DMAs. sync (HWDGE) carries the critical path in the order
        #    the PE consumes it: wq, xk0, wk, xk1..xk7. gpsimd (SWDGE, slower)
        #    carries everything not needed until the scores/v phases.
        wq_sb = pp.tile([128, KC, DPC], bf16, name="wq_sb")
        nc.sync.dma_start(
            out=wq_sb, in_=wqL[:, :].rearrange("p (kc m) -> p kc m", kc=KC)
        )
        xT_re = xT[:, :].rearrange("(kc p) n -> p kc n", p=128)
        xT_sb = []
        for kc in range(KC):
            xk = pp.tile([128, S], bf16, name=f"xk{kc}")
            xT_sb.append(xk)
        # x alternates between the sync and scalar HWDGE rings: one ring tops
        # out ~230-270 GB/s, two get the stream closer to the HBM limit. The
        # scalar engine is idle until the projection copies ~15us later. wk
        # goes down the scalar ring first so both weights land in parallel.
        wk_sb = pp.tile([128, KC, DPC], bf16, name="wk_sb")
        nc.scalar.dma_start(
            out=wk_sb, in_=wkL[:, :].rearrange("p (kc m) -> p kc m", kc=KC)
        )
        nc.sync.dma_start(out=xT_sb[0], in_=xT_re[:, 0, :])
        for kc in range(1, KC):
            eng = nc.scalar if kc % 2 else nc.sync
            eng.dma_start(out=xT_sb[kc], in_=xT_re[:, kc, :])
        # wv/wo ride the same HWDGE queue AFTER the x stream: the slow SWDGE
        # queue contends with x for DMA-engine slots if used this early
        w_sbs = {"q": wq_sb, "k": wk_sb}
        w_sbs["v"] = pp.tile([128, KC, DPC], bf16, name="wv_sb")
        nc.sync.dma_start(
            out=w_sbs["v"], in_=wvL[:, :].rearrange("p (kc m) -> p kc m", kc=KC)
        )
        wo_sb = pp.tile([128, HID], bf16)
        nc.sync.dma_start(out=wo_sb, in_=woT[:, :])

        # qU / kR: per head, 67 live rows ([0:64] proj, [64:67] bias factors)
        qU = [pp.tile([67, S], bf16, name=f"qU{h}") for h in range(HPC)]
        kR = [pp.tile([67, S], bf16, name=f"kR{h}") for h in range(HPC)]
        for h in range(HPC):
            nc.gpsimd.dma_start(out=qU[h][64:67, :], in_=uT[0:3, :])
            nc.gpsimd.dma_start(out=kR[h][64:67, :], in_=rT[0:3, :])
        if has_bq:
            bq_sb = pp.tile([DPC, 1], fp32)
            nc.gpsimd.dma_start(out=bq_sb, in_=bq[:, :])
        if has_bk:
            bk_sb = pp.tile([DPC, 1], fp32)
            nc.gpsimd.dma_start(out=bk_sb, in_=bk[:, :])
        if has_bv:
            bv_sb = pp.tile([128, DPC], fp32)
            bv_ap = bv[:, :]
            nc.gpsimd.dma_start(
                out=bv_sb,
                in_=bass.AP(tensor=bv_ap.tensor, offset=bv_ap.offset,
                            ap=[[0, 128], bv_ap.ap[1]]),
            )

        # v': [128, jb, 2 heads, 65] with ones in column 64 (one strided memset)
        vp = pp.tile([128, NJB, HPC, 65], bf16, name="vp")
        nc.gpsimd.memset(vp[:, :, :, 64:65], 1.0)
        # normalized transposed context, both heads, one tile per i-chunk
        ctxT = [pp.tile([128, 512], bf16, name=f"ctxT{ic}") for ic in range(NIC)]
        # staircase causal mask for a diagonal 128-col strip: keep iff f >= p
        stair = None
        if causal:
            stair = pp.tile([128, 128], bf16)
            nc.vector.memset(stair, 1.0)
            nc.gpsimd.affine_select(
                out=stair, in_=stair,
                compare_op=mybir.AluOpType.is_ge,
                fill=0.0, base=0,
                pattern=[[1, 128]],
                channel_multiplier=-1,
            )
            stair_b2 = bass.AP(
                tensor=stair.tensor, offset=stair.offset,
                ap=[stair.ap[0], [0, HPC], stair.ap[1]],
            )
        # ------- PSUM plan: 8 banks as 8 [128,512] slots.
        #   A,B: ctx chains (h0,h1)   C,D: v / outproj / bcast   S00..S11:
        #   per-(head,parity) score tiles. The qk mega-batch briefly uses all 8.
        p2 = ctx.enter_context(tc.tile_pool(name="ph2", bufs=1))
        ps = ctx.enter_context(tc.tile_pool(name="ps", bufs=1, space="PSUM"))
        at_tiles = {}
        ctx_cps = {}
        qk_ps = {}

        # q chains take the four [128,512] slots; k chains accumulate into
        # halves of the two [128,1024] score-tile buffers, which the first
        # score chunks then rotate into (their freeing copies run first on V)
        QKSLOT = {("q", 0): "A", ("q", 1): "B", ("q", 2): "C", ("q", 3): "D"}

        def emit_qk_mega():
            # kc-outer over all 8 chains so matmuls start as soon as each x
            # chunk lands; q chains first so the wk DMA wait is hidden
            chains = [("q", n) for n in range(NIC)] + [("k", n) for n in range(NIC)]
            for n in range(NIC):
                qk_ps[("q", n)] = ps.tile([128, 512], fp32,
                                          tag=QKSLOT[("q", n)], name=f"ps_q{n}")
            for half in range(2):
                kk = ps.tile([128, HPC * 512], fp32, tag="sc", bufs=2,
                             name=f"ps_kk{half}")
                qk_ps[("k", 2 * half)] = kk[:, 0:512]
                qk_ps[("k", 2 * half + 1)] = kk[:, 512:1024]
            for kc in range(KC):
                for nm, n in chains:
                    nc.tensor.matmul(
                        qk_ps[(nm, n)],
                        lhsT=w_sbs[nm][:, kc, :],
                        rhs=xT_sb[kc][:, n * 512:(n + 1) * 512],
                        start=(kc == 0),
                        stop=(kc == KC - 1),
                    )

        def emit_copies(nm, n, eng="vec"):
            # PSUM -> qU/kR, spread across scalar/vector/gpsimd so the
            # 16-copy burst after the projection batch drains ~3x faster
            dsts = qU if nm == "q" else kR
            bias_sb = None
            if nm == "q" and has_bq:
                bias_sb = bq_sb
            if nm == "k" and has_bk:
                bias_sb = bk_sb
            for h in range(HPC):
                dst = dsts[h][0:64, n * 512:(n + 1) * 512]
                sr = qk_ps[(nm, n)][h * 64:(h + 1) * 64, :]
                if bias_sb is not None:
                    nc.vector.tensor_scalar_add(
                        dst, sr, bias_sb[h * 64:(h + 1) * 64, 0:1]
                    )
                elif eng == "act":
                    nc.scalar.activation(dst, sr, Copy)
                else:
                    nc.vector.tensor_copy(dst, sr)

        def emit_v(vjb, slots="CD"):
            vtag = slots[vjb % 2]
            psv = ps.tile([128, DPC], fp32, tag=vtag, name=f"psv{vjb}")
            for kc in range(KC):
                nc.tensor.matmul(
                    psv,
                    lhsT=xT_sb[kc][:, vjb * 128:(vjb + 1) * 128],
                    rhs=w_sbs["v"][:, kc, :],
                    start=(kc == 0),
                    stop=(kc == KC - 1),
                )
            dst = vp[:, vjb, :, 0:64]
            sr = psv[:, :].rearrange("p (h m) -> p h m", h=HPC)
            if has_bv:
                bvr = bv_sb[:, :].rearrange("p (h m) -> p h m", h=HPC)
                nc.vector.tensor_add(dst, sr, bvr)
            else:
                nc.vector.tensor_copy(dst, sr)

        def emit_chunk(jb, ic):
            if causal:
                ics = (jb * 128) // 512
                w = S - ics * 512
                key = jb
            else:
                ics, w, key = ic, 512, (jb, ic)
            if key not in at_tiles:
                at_tiles[key] = p2.tile(
                    [128, HPC, w], bf16, tag=f"at{jb}",
                    bufs=1 if causal else 2, name=f"at{jb}_{ic}")
            at = at_tiles[key]
            diag = causal and ic == ics
            d0 = (jb % 4) * 128 if diag else 0
            off = (ic - ics) * 512
            sc = ps.tile([128, HPC * 512], fp32, tag="sc", bufs=2,
                         name=f"sc{jb}_{ic}")
            for h in range(HPC):
                nc.tensor.matmul(
                    sc[:, h * 512 + d0:(h + 1) * 512],
                    lhsT=kR[h][:, jb * 128:(jb + 1) * 128],
                    rhs=qU[h][:, ic * 512 + d0:(ic + 1) * 512],
                    start=True,
                    stop=True,
                )
            scr = sc[:, :].rearrange("p (h n) -> p h n", h=HPC)
            nc.scalar.activation(
                at[:, :, off + d0:off + 512], scr[:, :, d0:], Exp
            )
            if diag:
                # NB: the stair multiply must stay on vector — a gpsimd
                # TENSOR_TENSOR forces ucode library swaps against
                # partition_broadcast (~7us UNLOAD/LOAD stall per phase)
                if d0:
                    nc.gpsimd.memset(at[:, :, 0:d0], 0.0)
                nc.vector.tensor_mul(
                    at[:, :, d0:d0 + 128], at[:, :, d0:d0 + 128], stair_b2
                )
            if not causal:
                mt = p2.tile([128, 512], bf16, tag="mt", bufs=2,
                             name=f"mt{jb}_{ic}")
                nc.sync.dma_start(
                    out=mt,
                    in_=maskT[jb * 128:(jb + 1) * 128,
                              ic * 512:(ic + 1) * 512])
                mt_b2 = bass.AP(
                    tensor=mt.tensor, offset=mt.offset,
                    ap=[mt.ap[0], [0, HPC], mt.ap[1]],
                )
                nc.vector.tensor_mul(at, at, mt_b2)

        def emit_ctx_steps(h, ic, jbs):
            # incremental attn@V accumulation; the PSUM group stays open
            # across other banks' matmuls, so these trail the exp stream
            jmax = (ic + 1) * 4 if causal else NJB
            if (h, ic) not in ctx_cps:
                ctx_cps[(h, ic)] = ps.tile([65, 512], fp32, tag="AB"[h],
                                           name=f"cps{h}_{ic}")
            cps = ctx_cps[(h, ic)]
            for jb in jbs:
                if causal:
                    at = at_tiles[jb]
                    ics = (jb * 128) // 512
                    rhs = at[:, h, (ic - ics) * 512:(ic - ics + 1) * 512]
                else:
                    rhs = at_tiles[(jb, ic)][:, h, 0:512]
                nc.tensor.matmul(
                    cps,
                    lhsT=vp[:, jb, h, :],
                    rhs=rhs,
                    start=(jb == 0),
                    stop=(jb == jmax - 1),
                )

        norm_tiles = {}

        def emit_norm(h, ic, cols=(0, 512), tail=False):
            # den row out of PSUM (custom-DVE ops misread PSUM on hw, so the
            # copy stays), reciprocal on the small row, broadcast, multiply.
            # The tail runs this in column halves so the first out-projection
            # unblocks ~2x sooner; copies ride the idle scalar engine there.
            cps = ctx_cps[(h, ic)]
            lo, hi = cols
            if (h, ic) not in norm_tiles:
                rr = p2.tile([1, 512], fp32, tag="rr", bufs=2,
                             name=f"rr{h}_{ic}")
                rr2 = p2.tile([1, 512], fp32, tag="rr2", bufs=2,
                              name=f"rr2{h}_{ic}")
                rb = p2.tile([64, 512], fp32, tag="rb", bufs=2,
                             name=f"rb{h}_{ic}")
                norm_tiles[(h, ic)] = (rr, rr2, rb)
            rr, rr2, rb = norm_tiles[(h, ic)]
            if tail:
                nc.scalar.activation(rr[:, lo:hi], cps[64:65, lo:hi], Copy)
            else:
                nc.vector.tensor_copy(rr[:, lo:hi], cps[64:65, lo:hi])
            nc.vector.reciprocal_approx_fast(rr2[:, lo:hi], rr[:, lo:hi])
            nc.gpsimd.partition_broadcast(rb[:, lo:hi], rr2[:, lo:hi])
            nc.vector.tensor_mul(
                ctxT[ic][h * 64:(h + 1) * 64, lo:hi], cps[0:64, lo:hi],
                rb[:, lo:hi],
            )

        def emit_outproj(ib, tail=False, slots="CD"):
            ob = p2.tile([128, HID], bf16, tag="ob", bufs=3, name=f"ob{ib}")
            for oc in range(2):
                ops = ps.tile([128, 512], fp32, tag=slots[oc],
                              name=f"ops{ib}_{oc}")
                nc.tensor.matmul(
                    ops,
                    lhsT=ctxT[ib // 4][:, (ib % 4) * 128:(ib % 4 + 1) * 128],
                    rhs=wo_sb[:, oc * 512:(oc + 1) * 512],
                    start=True,
                    stop=True,
                )
                # early out-copies stay off ACT (its exp stream is the
                # critical path mid-kernel); tail ibs use idle ACT for oc0
                if tail and oc == 0:
                    nc.scalar.activation(ob[:, oc * 512:(oc + 1) * 512],
                                         ops, Copy)
                else:
                    nc.vector.tensor_copy(ob[:, oc * 512:(oc + 1) * 512], ops)
            nc.sync.dma_start(out=out[ib * 128:(ib + 1) * 128, :], in_=ob)

        if causal:
            emit_qk_mega()
            # ph0. Copy streams: k0-k3 on vector, q0/q1 on scalar, q2/q3 on
            # gpsimd — three engines drain the 16-copy burst concurrently.
            # Chunks are emitted before later copies of the same qU/kR tiles
            # (readers wait on every prior-emitted writer of a tile), and v
            # chains (which need no copies) fill the PE while copies drain.
            emit_copies("k", 0)
            emit_copies("k", 1)
            emit_copies("q", 0, "act")
            emit_copies("q", 1, "act")
            emit_chunk(0, 0)
            emit_v(0, "AB")
            emit_v(1, "AB")
            emit_copies("k", 2)
            emit_copies("k", 3)
            emit_copies("q", 2, "act")
            emit_copies("q", 3, "act")
            emit_v(2)
            emit_v(3)
            emit_chunk(1, 0)
            emit_v(4)
            emit_v(5)
            emit_chunk(2, 0)
            emit_v(6)
            emit_v(7)
            emit_chunk(3, 0)
            emit_v(8, "AB")
            emit_v(9, "AB")
            for h in range(HPC):
                emit_ctx_steps(h, 0, [0, 1, 2, 3])
            emit_norm(0, 0)
            emit_norm(1, 0)
            # ph1: scores ic1; v10-13; ctx(.,1) trails; outproj 0-3
            emit_chunk(0, 1)
            emit_chunk(1, 1)
            emit_v(10)
            emit_chunk(2, 1)
            emit_v(11)
            emit_chunk(3, 1)
            emit_v(12)
            emit_chunk(4, 1)
            emit_v(13)
            emit_ctx_steps(0, 1, [0, 1])
            emit_ctx_steps(1, 1, [0, 1])
            emit_chunk(5, 1)
            emit_ctx_steps(0, 1, [2, 3])
            emit_ctx_steps(1, 1, [2, 3])
            emit_outproj(0)
            emit_chunk(6, 1)
            emit_ctx_steps(0, 1, [4, 5])
            emit_ctx_steps(1, 1, [4, 5])
            emit_outproj(1)
            emit_chunk(7, 1)
            emit_ctx_steps(0, 1, [6])
            emit_ctx_steps(1, 1, [6])
            emit_outproj(2)
            emit_outproj(3)
            emit_ctx_steps(0, 1, [7])
            emit_ctx_steps(1, 1, [7])
            emit_norm(0, 1)
            emit_norm(1, 1)
            # ph2: scores ic2; v14-15; ctx(.,2) trails; outproj 4-7
            emit_chunk(0, 2)
            emit_chunk(1, 2)
            emit_v(14)
            emit_chunk(2, 2)
            emit_v(15)
            emit_chunk(3, 2)
            emit_chunk(4, 2)
            emit_ctx_steps(0, 2, [0, 1])
            emit_ctx_steps(1, 2, [0, 1])
            emit_chunk(5, 2)
            emit_ctx_steps(0, 2, [2, 3])
            emit_ctx_steps(1, 2, [2, 3])
            emit_outproj(4)
            emit_chunk(6, 2)
            emit_ctx_steps(0, 2, [4])
            emit_ctx_steps(1, 2, [4])
            emit_chunk(7, 2)
            emit_ctx_steps(0, 2, [5])
            emit_ctx_steps(1, 2, [5])
            emit_outproj(5)
            emit_chunk(8, 2)
            emit_ctx_steps(0, 2, [6])
            emit_ctx_steps(1, 2, [6])
            emit_chunk(9, 2)
            emit_ctx_steps(0, 2, [7])
            emit_ctx_steps(1, 2, [7])
            emit_outproj(6)
            emit_chunk(10, 2)
            emit_ctx_steps(0, 2, [8])
            emit_ctx_steps(1, 2, [8])
            emit_chunk(11, 2)
            emit_ctx_steps(0, 2, [9])
            emit_ctx_steps(1, 2, [9])
            emit_outproj(7)
            emit_ctx_steps(0, 2, [10])
            emit_ctx_steps(1, 2, [10])
            emit_ctx_steps(0, 2, [11])
            emit_ctx_steps(1, 2, [11])
            emit_norm(0, 2)
            emit_norm(1, 2)
            # ph3: scores ic3; ctx(.,3) trails TWO chunks behind (the exp
            # stream runs ~1us behind the score matmuls); outproj 8-11 late
            emit_chunk(0, 3)
            emit_chunk(1, 3)
            emit_chunk(2, 3)
            emit_chunk(3, 3)
            emit_ctx_steps(0, 3, [0, 1])
            emit_ctx_steps(1, 3, [0, 1])
            for jb in range(4, 8):
                emit_chunk(jb, 3)
                emit_ctx_steps(0, 3, [jb - 2])
                emit_ctx_steps(1, 3, [jb - 2])
            emit_chunk(8, 3)
            emit_ctx_steps(0, 3, [6])
            emit_ctx_steps(1, 3, [6])
            emit_chunk(9, 3)
            emit_ctx_steps(0, 3, [7])
            emit_ctx_steps(1, 3, [7])
            emit_chunk(10, 3)
            emit_ctx_steps(0, 3, [8])
            emit_ctx_steps(1, 3, [8])
            emit_outproj(8)
            emit_chunk(11, 3)
            emit_ctx_steps(0, 3, [9])
            emit_ctx_steps(1, 3, [9])
            emit_chunk(12, 3)
            emit_ctx_steps(0, 3, [10])
            emit_ctx_steps(1, 3, [10])
            emit_outproj(9)
            emit_chunk(13, 3)
            emit_ctx_steps(0, 3, [11])
            emit_ctx_steps(1, 3, [11])
            emit_chunk(14, 3)
            emit_ctx_steps(0, 3, [12])
            emit_ctx_steps(1, 3, [12])
            emit_outproj(10)
            emit_chunk(15, 3)
            emit_ctx_steps(0, 3, [13])
            emit_ctx_steps(1, 3, [13])
            emit_outproj(11)
            emit_ctx_steps(0, 3, [14])
            emit_ctx_steps(1, 3, [14])
            emit_ctx_steps(0, 3, [15])
            emit_ctx_steps(1, 3, [15])
            # tail: 128-col norm quarters, each matched to exactly the ctxT
            # columns its out-projection consumes, so outproj ib unblocks as
            # soon as quarter ib-12 is normalized; each tail outproj gets its
            # own pair of PSUM slots (A/B free once the norm muls have read
            # the ctx chains)
            # A/B only usable by the LAST outproj: earlier ones would clobber
            # the cps tiles while later norm quarters still read them
            for quarter, ib, slots in ((0, 12, "CD"), (1, 13, "CD"),
                                       (2, 14, "CD"), (3, 15, "AB")):
                cols = (quarter * 128, (quarter + 1) * 128)
                emit_norm(0, 3, cols=cols, tail=True)
                emit_norm(1, 3, cols=cols, tail=True)
                emit_outproj(ib, tail=True, slots=slots)
        else:
            emit_qk_mega()
            for n in range(NIC):
                emit_copies("k", n)
            for n in range(NIC):
                emit_copies("q", n)
            for vjb in range(NJB):
                emit_v(vjb)
            for ic in range(NIC):
                for jb in range(NJB):
                    emit_chunk(jb, ic)
                    if jb >= 1:
                        emit_ctx_steps(0, ic, [jb - 1])
                        emit_ctx_steps(1, ic, [jb - 1])
                emit_ctx_steps(0, ic, [NJB - 1])
                emit_ctx_steps(1, ic, [NJB - 1])
                emit_norm(0, ic, tail=(ic == NIC - 1))
                emit_norm(1, ic, tail=(ic == NIC - 1))
                for ib in range(4 * ic, 4 * (ic + 1)):
                    emit_outproj(ib, tail=(ic == NIC - 1))

    nc.compile()
    return nc


def kernel(x, Wq, bq, Wk, bk, Wv, bv, Wo, bo, cmw, mask, modality_info,
           _perf=None):
    from concourse.bass_utils import run_bass_kernel_spmd

    x = np.asarray(x, np.float32)
    Wq = np.asarray(Wq, np.float32)
    Wk = np.asarray(Wk, np.float32)
    Wv = np.asarray(Wv, np.float32)
    Wo = np.asarray(Wo, np.float32)
    bq_ = np.asarray(bq, np.float32)
    bk_ = np.asarray(bk, np.float32)
    bv_ = np.asarray(bv, np.float32)
    bo_ = np.asarray(bo, np.float32)
    cmw = np.asarray(cmw, np.float32)
    mask2 = np.asarray(mask)[0]
    mi = np.asarray(modality_info).astype(np.int64)[0]

    causal = bool(
        np.array_equal(mask2 != 0, np.tril(np.ones((S, S), bool)))
    )
    has_bq = bool(np.any(bq_))
    has_bk = bool(np.any(bk_))
    has_bv = bool(np.any(bv_))

    key = (causal, has_bq, has_bk, has_bv)
    if key not in _CACHE:
        _CACHE[key] = _build(*key)
    nc = _CACHE[key]

    scale = 1.0 / math.sqrt(D)
    # rank-3 factorization of the gathered cross-modal bias
    R = np.zeros((S, 3), np.float32)
    R[np.arange(S), mi] = 1.0
    U = R @ cmw
    uT4 = np.zeros((4, S), BF16)
    rT4 = np.zeros((4, S), BF16)
    uT4[0:3, :] = U.T.astype(BF16)
    rT4[0:3, :] = R.T.astype(BF16)
    xTb = np.ascontiguousarray(x[0].T).astype(BF16)

    def wlayout(wT):
        # [HID, DPC] -> [128, KC*DPC] with row p = wT[kc*128+p, :] for all kc
        return np.ascontiguousarray(
            wT.reshape(KC, 128, DPC).transpose(1, 0, 2).reshape(128, KC * DPC)
        ).astype(BF16)

    in_maps = []
    for c in range(NCORES):
        sl = slice(c * DPC, (c + 1) * DPC)
        m = {
            "xT": xTb,
            # scores scale folded into the q-side weights (and bias)
            "wqL": wlayout(Wq[sl, :].T * scale),
            "wkL": wlayout(Wk[sl, :].T),
            "wvL": wlayout(Wv[sl, :].T),
            "woT": np.ascontiguousarray(Wo[:, sl].T).astype(BF16),
            "uT": uT4,
            "rT": rT4,
        }
        if has_bq:
            m["bq"] = np.ascontiguousarray(bq_[sl, None] * scale)
        if has_bk:
            m["bk"] = np.ascontiguousarray(bk_[sl, None])
        if has_bv:
            m["bv"] = np.ascontiguousarray(bv_[None, sl])
        if not causal:
            m["maskT"] = np.ascontiguousarray(mask2.T != 0).astype(BF16)
        in_maps.append(m)

    res = run_bass_kernel_spmd(
        nc, in_maps, core_ids=list(range(NCORES)),
        trace=bool(_perf is not None),
    )
    outp = np.zeros((S, HID), np.float32)
    for r in res.results:
        outp += np.asarray(r["out"], dtype=np.float32)
    outp += bo_[None, :]
    if _perf is not None:
        _perf["exec_time_ns"] = res.exec_time_ns
        _perf["trace"] = res.instructions_and_trace
    return outp.reshape(B, S, HID)

